# revision 16
# baseline (speedup 1.0000x reference)
"""Trainium2 Bass kernel for nn_CalcImpute (retrieval KNN imputation).

Computes, per row r of dist_pot_donors [8192, 32768]:
  - the 16 smallest distances (ties broken by lowest column index, matching
    jax.lax.top_k on the negated matrix),
  - inverse-distance weights (with sklearn-style handling of exact-zero
    distances: rows containing a zero distance use {0,1} weights),
  - masked weighted mean of fit_X_col at the selected indices.

Sharding: rows are data-parallel across 8 NeuronCores (1024 rows each);
fit_X_col / mask_fit_X_col are replicated (combined into one small table).

Device algorithm (per core, per block of 128 rows on partitions):
  1. Stream the block's [128, 32768] distances in 4 panels, cast f32->bf16
     during the DMA (SWDGE; HBM read traffic unchanged, SBUF tile halved).
     Chunk minima of 2048 16-element column chunks via a DVE pairwise-min
     tree in bf16 (tensor_tensor runs in 2x_1P mode for 16-bit dtypes —
     DVE tensor_reduce has no fast mode, so the bf16 tree is ~2x faster
     than an f32 reduce), negated by a cheap 4x-mode tensor_scalar.
  2. Select the top-24 chunks per row by (bf16 chunk_min asc, chunk_idx
     asc) via 3x fp-max8/max_index + 2x match_replace.  24 >= 16 chunks
     guarantees the true top-16 ELEMENTS (by exact f32 value) live in the
     gathered pool: any chunk holding a top-16 element has bf16-min <=
     bf16(V); more than 24 such chunks would need ~9 distinct chunk
     minima inside one bf16 ulp of V.  The dense exact f32 selection in
     step 4 then reproduces the reference bit-level selection.
  3. ONE batched indirect DMA per table per block regathers the 24
     winning chunks of all 128 rows in exact f32 (offset ap [128,24] ->
     3072 descriptors in a single instruction) — and likewise the
     combined xm/mb donor table at the same chunk ids.
  4. Dense exact selection on the gathered [128, 384] f32 values:
     boundary value V = 16th smallest; elements < V always selected;
     elements == V selected in ascending global-index order until 16 —
     implemented arithmetically, spread across DVE/GPSIMD/ACT (the Pool
     engine only supports tensor_scalar / tensor_tensor{add,sub,mult} /
     tensor_copy, so compares-vs-scalar and products go there).
  5. Weights 1/d (reciprocal_approx_accurate, ~2 ULP) with zero-distance
     row fixup, masked weighted mean; per-block results accumulate in
     SBUF and are written out with a single DMA at the end.

Assumptions: no NaNs in the distance matrix (inputs are uniform [0,1)),
n_neighbors == 16.  Everything else (ties, exact zeros, all-masked rows)
is handled exactly.
"""

from contextlib import ExitStack

import numpy as np

import concourse.bacc as bacc
import concourse.bass as bass
import concourse.mybir as mybir
import concourse.tile as tile
from concourse import bass_utils

F32 = mybir.dt.float32
BF16 = mybir.dt.bfloat16
I32 = mybir.dt.int32
U32 = mybir.dt.uint32
ALU = mybir.AluOpType
ACTF = mybir.ActivationFunctionType

R_FULL = 8192          # total rows
D = 32768              # donors (columns)
K = 16                 # n_neighbors
KP = 24                # gathered chunks per row (margin over K for bf16 L1)
N_CORES = 8
RPC = R_FULL // N_CORES  # rows per core (1024)
P = 128                # partitions
NB = RPC // P          # row blocks per core (8)
NPAN = 4               # column panels per block
W = D // NPAN          # panel width (8192)
CH = 16                # chunk length
NCHUNK = D // CH       # chunks per row (2048)
CPP = W // CH          # chunks per panel (512)
GW = KP * CH           # gathered width per row (384)
NEG_SENT = -3.0e38


def build_module(rep: int = 1, stages: str = "full", small_input: bool = False,
                 debug_dump: bool = False):
    """Build the Bass module. rep>1 wraps the compute body in an on-device
    For_i loop (for timing: marginal cost per rep = true kernel time, fixed
    overheads like the axon upload cancel).

    stages: "full" | "l1" | "l2" | "gather" | "l3ng" — ablation levels for
    timing attribution ("l3ng" = level-3 math with memset instead of
    gathers).

    small_input: timing-only — d input is a single 16 MiB block read 8x per
    rep (64x smaller upload; faithful HBM/compute behavior)."""
    do_l2 = stages in ("l2", "gather", "l3ng", "full")
    do_gather = stages in ("gather", "full")
    do_l3 = stages in ("l3ng", "full")
    l2_scans = {"l2a": 1, "l2b": 2}.get(stages, 0)
    nc = bacc.Bacc("TRN2", target_bir_lowering=False, debug=False)

    d_rows = P if small_input else RPC
    d_dram = nc.dram_tensor("d", (d_rows, D), F32, kind="ExternalInput")
    comb_dram = nc.dram_tensor("comb", (d_rows * NCHUNK, 3 * CH), F32,
                               kind="ExternalInput")
    res_dram = nc.dram_tensor("res", (RPC, 1), F32, kind="ExternalOutput")

    dbg = {}
    if debug_dump:
        for name, shape in [("negmins_dbg", (P, NCHUNK)),
                            ("cidx_dbg", (P, KP)),
                            ("G_dbg", (P, KP * 3 * CH)),
                            ("X_dbg", (P, GW)),
                            ("sel_dbg", (P, GW)),
                            ("wsel_dbg", (P, GW))]:
            dbg[name] = nc.dram_tensor(name, shape, F32,
                                       kind="ExternalOutput")

    with tile.TileContext(nc) as tc:
        with ExitStack() as ctx:
            const = ctx.enter_context(tc.tile_pool(name="const", bufs=1))
            dpool = ctx.enter_context(tc.tile_pool(name="dpool", bufs=3))
            gpool = ctx.enter_context(tc.tile_pool(name="gpool", bufs=2))
            mpool = ctx.enter_context(tc.tile_pool(name="mpool", bufs=2))
            spool = ctx.enter_context(tc.tile_pool(name="spool", bufs=2))

            # ---- constants ----
            iota_t_i = const.tile([P, GW], I32, tag="iota_t_i")
            nc.gpsimd.iota(iota_t_i[:].rearrange("p (c t) -> p c t", t=CH),
                           pattern=[[0, KP], [1, CH]], base=0,
                           channel_multiplier=0)
            iota_t_f = const.tile([P, GW], F32, tag="iota_t_f")
            nc.vector.tensor_copy(iota_t_f[:], iota_t_i[:])

            iota16_i = const.tile([P, K], I32, tag="iota16_i")
            nc.gpsimd.iota(iota16_i[:], pattern=[[1, K]], base=0,
                           channel_multiplier=0)
            iota16_f = const.tile([P, K], F32, tag="iota16_f")
            nc.vector.tensor_copy(iota16_f[:], iota16_i[:])

            # per-partition row index (within block) * NCHUNK
            rowb_i = const.tile([P, 1], I32, tag="rowb_i")
            nc.gpsimd.iota(rowb_i[:], pattern=[[0, 1]], base=0,
                           channel_multiplier=NCHUNK)
            rowb_f = const.tile([P, 1], F32, tag="rowb_f")
            nc.vector.tensor_copy(rowb_f[:], rowb_i[:])

            # per-block results accumulate here; one DMA out at the end
            res_acc = const.tile([P, NB], F32, tag="res_acc")

            loop_ctx = tc.For_i(0, rep, 1) if rep > 1 else None
            if loop_ctx is not None:
                loop_ctx.__enter__()
            if True:
                for b in range(NB):
                    # ---------- level 1: negated bf16 chunk minima ----------
                    negmins = mpool.tile([P, NCHUNK], BF16, tag="negmins")
                    for p in range(NPAN):
                        dt = dpool.tile([P, CPP, CH], BF16, tag="dt")
                        rb = 0 if small_input else b * P
                        # SWDGE cast-DMA: read f32 from HBM, write bf16
                        nc.gpsimd.dma_start(
                            dt[:].rearrange("p c t -> p (c t)"),
                            d_dram.ap()[rb:rb + P, p * W:(p + 1) * W])
                        # bf16 pairwise-min tree (2x_1P on rounds 1-3)
                        s1 = gpool.tile([P, CPP, 8], BF16, tag="s1")
                        nc.vector.tensor_tensor(
                            s1[:], dt[:, :, 0:8], dt[:, :, 8:16], op=ALU.min)
                        s2 = gpool.tile([P, CPP, 4], BF16, tag="s2")
                        nc.vector.tensor_tensor(
                            s2[:], s1[:, :, 0:4], s1[:, :, 4:8], op=ALU.min)
                        s3 = gpool.tile([P, CPP, 2], BF16, tag="s3")
                        nc.vector.tensor_tensor(
                            s3[:], s2[:, :, 0:2], s2[:, :, 2:4], op=ALU.min)
                        s4 = gpool.tile([P, CPP], BF16, tag="s4")
                        nc.vector.tensor_tensor(
                            s4[:], s3[:, :, 0], s3[:, :, 1], op=ALU.min)
                        # negate (4x-mode tensor_scalar)
                        nc.vector.tensor_scalar(
                            negmins[:, p * CPP:(p + 1) * CPP], s4[:],
                            -1.0, None, op0=ALU.mult)

                    if debug_dump and b == 0:
                        nc.gpsimd.dma_start(dbg["negmins_dbg"].ap(),
                                            negmins[:])

                    if l2_scans:
                        # timing ablations: l2a = 3x max8 only;
                        # l2b = max8/mr interleave (no max_index)
                        t8a = spool.tile([P, 8], BF16, tag="t8a")
                        nc.vector.max(t8a[:], negmins[:])
                        if l2_scans == 1:
                            u1 = spool.tile([P, 8], BF16, tag="u1")
                            nc.vector.max(u1[:], negmins[:])
                            u2 = spool.tile([P, 8], BF16, tag="u2")
                            nc.vector.max(u2[:], negmins[:])
                            last = u2
                        else:
                            nm2 = mpool.tile([P, NCHUNK], BF16, tag="negmins2")
                            nc.vector.match_replace(nm2[:], t8a[:],
                                                    negmins[:], NEG_SENT)
                            t8b = spool.tile([P, 8], BF16, tag="t8b")
                            nc.vector.max(t8b[:], nm2[:])
                            nm3 = mpool.tile([P, NCHUNK], BF16, tag="negmins3")
                            nc.vector.match_replace(nm3[:], t8b[:],
                                                    nm2[:], NEG_SENT)
                            t8c = spool.tile([P, 8], BF16, tag="t8c")
                            nc.vector.max(t8c[:], nm3[:])
                            last = t8c
                        res_b = spool.tile([P, 1], F32, tag="res_b")
                        nc.vector.tensor_copy(res_b[:], last[:, 0:1])
                        nc.vector.tensor_copy(res_acc[:, b:b + 1], res_b[:])
                        continue

                    if not do_l2:
                        res_b = spool.tile([P, 1], F32, tag="res_b")
                        nc.vector.tensor_copy(res_b[:], negmins[:, 0:1])
                        nc.vector.tensor_copy(res_acc[:, b:b + 1], res_b[:])
                        continue

                    # ---------- level 2: top-24 chunks per row ----------
                    t8a = spool.tile([P, 8], BF16, tag="t8a")
                    nc.vector.max(t8a[:], negmins[:])
                    pos_a = spool.tile([P, 8], U32, tag="pos_a")
                    nc.vector.max_index(pos_a[:], t8a[:], negmins[:])
                    negmins2 = mpool.tile([P, NCHUNK], BF16, tag="negmins2")
                    nc.vector.match_replace(negmins2[:], t8a[:], negmins[:],
                                            NEG_SENT)
                    t8b = spool.tile([P, 8], BF16, tag="t8b")
                    nc.vector.max(t8b[:], negmins2[:])
                    pos_b = spool.tile([P, 8], U32, tag="pos_b")
                    nc.vector.max_index(pos_b[:], t8b[:], negmins2[:])
                    negmins3 = mpool.tile([P, NCHUNK], BF16, tag="negmins3")
                    nc.vector.match_replace(negmins3[:], t8b[:], negmins2[:],
                                            NEG_SENT)
                    t8c = spool.tile([P, 8], BF16, tag="t8c")
                    nc.vector.max(t8c[:], negmins3[:])
                    pos_c = spool.tile([P, 8], U32, tag="pos_c")
                    nc.vector.max_index(pos_c[:], t8c[:], negmins3[:])

                    cidx_f = spool.tile([P, KP], F32, tag="cidx_f")
                    nc.vector.tensor_copy(cidx_f[:, 0:8], pos_a[:])
                    nc.vector.tensor_copy(cidx_f[:, 8:16], pos_b[:])
                    nc.vector.tensor_copy(cidx_f[:, 16:24], pos_c[:])

                    # zero-distance row indicator: t8a[0] = -(row min), bf16
                    # rounding is exact at 0.
                    zr = spool.tile([P, 1], F32, tag="zr")
                    nc.vector.tensor_scalar(zr[:], t8a[:, 0:1], 0.0, None,
                                            op0=ALU.is_equal)
                    zc = spool.tile([P, 1], F32, tag="zc")
                    nc.vector.tensor_scalar(zc[:], zr[:], -1.0, 1.0,
                                            op0=ALU.mult, op1=ALU.add)

                    # chunk ids -> shard-global gather indices (f32 then i32)
                    rowbase_b = spool.tile([P, 1], F32, tag="rowbase_b")
                    rbase = 0.0 if small_input else float(b * P * NCHUNK)
                    nc.vector.tensor_scalar(rowbase_b[:], rowb_f[:],
                                            rbase, None, op0=ALU.add)
                    gidx_f = spool.tile([P, KP], F32, tag="gidx_f")
                    nc.vector.tensor_scalar(gidx_f[:], cidx_f[:],
                                            rowbase_b[:], None, op0=ALU.add)
                    gidx_i = spool.tile([P, KP], I32, tag="gidx_i")
                    nc.vector.tensor_copy(gidx_i[:], gidx_f[:])
                    cidx_i = spool.tile([P, KP], I32, tag="cidx_i")
                    nc.vector.tensor_copy(cidx_i[:], cidx_f[:])

                    # ---------- level 3: regather + exact selection ----------
                    GXB = spool.tile([P, KP, 3 * CH], F32, tag="GXB")
                    if do_gather:
                        # per-index gathers: one 192-B combined fetch per
                        # selected chunk (the multi-offset batched form is
                        # broken on HW - wrong descriptor order).
                        for j in range(KP):
                            nc.gpsimd.indirect_dma_start(
                                GXB[:, j, :], None, comb_dram.ap(),
                                bass.IndirectOffsetOnAxis(
                                    ap=gidx_i[:, j:j + 1], axis=0))
                    elif do_l3:
                        nc.vector.memset(GXB[:], 0.5)
                    if not do_l3:
                        res_b = spool.tile([P, 1], F32, tag="res_b")
                        nc.vector.tensor_scalar(res_b[:], cidx_f[:, 0:1],
                                                1.0, None, op0=ALU.mult)
                        nc.vector.tensor_copy(res_acc[:, b:b + 1], res_b[:])
                        continue
                    Gf3 = GXB[:, :, 0:CH]            # [P, KP, CH] strided
                    if debug_dump and b == 0:
                        nc.sync.dma_start(dbg["cidx_dbg"].ap(), cidx_f[:])
                        nc.sync.dma_start(
                            dbg["G_dbg"].ap(),
                            GXB[:].rearrange("p c t -> p (c t)"))

                    # global element index X per gathered slot  [gpsimd]
                    cid_b = cidx_f[:].unsqueeze(-1).broadcast_to((P, KP, CH))
                    X1 = spool.tile([P, GW], F32, tag="X1")
                    nc.gpsimd.tensor_scalar(
                        X1[:].rearrange("p (c t) -> p c t", t=CH),
                        cid_b, float(CH), None, op0=ALU.mult)
                    X = spool.tile([P, GW], F32, tag="X")
                    nc.gpsimd.tensor_tensor(X[:], X1[:], iota_t_f[:],
                                            op=ALU.add)

                    # negated gathered values  [ACT]
                    negG = spool.tile([P, GW], F32, tag="negG")
                    nc.scalar.activation(
                        negG[:].rearrange("p (c t) -> p c t", t=CH),
                        Gf3, ACTF.Copy, scale=-1.0)

                    # 16th smallest: nV = -(V)  [DVE]
                    g8a = spool.tile([P, 8], F32, tag="g8a")
                    nc.vector.max(g8a[:], negG[:])
                    negG2 = spool.tile([P, GW], F32, tag="negG2")
                    nc.vector.match_replace(negG2[:], g8a[:], negG[:], NEG_SENT)
                    g8b = spool.tile([P, 8], F32, tag="g8b")
                    nc.vector.max(g8b[:], negG2[:])
                    nV = g8b[:, 7:8]

                    # strictly-below mask + count  (negG > nV <=> Gf < V)
                    maskLT = spool.tile([P, GW], F32, tag="maskLT")
                    q = spool.tile([P, 1], F32, tag="q")
                    nc.vector.tensor_scalar(maskLT[:], negG[:], nV, None,
                                            op0=ALU.is_gt, op1=ALU.add,
                                            accum_out=q[:])
                    maskEQ = spool.tile([P, GW], F32, tag="maskEQ")
                    nc.gpsimd.tensor_scalar(maskEQ[:], negG[:], nV, None,
                                            op0=ALU.is_equal)

                    # ascending global indices of ==V elements (up to 16)
                    t1 = spool.tile([P, GW], F32, tag="t1")
                    nc.vector.tensor_scalar(t1[:], X[:], -1.0, 65536.0,
                                            op0=ALU.mult, op1=ALU.add)
                    t2 = spool.tile([P, GW], F32, tag="t2")
                    nc.gpsimd.tensor_tensor(t2[:], t1[:], maskEQ[:],
                                            op=ALU.mult)
                    e8a = spool.tile([P, 8], F32, tag="e8a")
                    nc.vector.max(e8a[:], t2[:])
                    t2b = spool.tile([P, GW], F32, tag="t2b")
                    nc.vector.match_replace(t2b[:], e8a[:], t2[:], NEG_SENT)
                    e8b = spool.tile([P, 8], F32, tag="e8b")
                    nc.vector.max(e8b[:], t2b[:])
                    xs = spool.tile([P, K], F32, tag="xs")
                    nc.vector.tensor_scalar(xs[:, 0:8], e8a[:], -1.0, 65536.0,
                                            op0=ALU.mult, op1=ALU.add)
                    nc.vector.tensor_scalar(xs[:, 8:16], e8b[:], -1.0, 65536.0,
                                            op0=ALU.mult, op1=ALU.add)

                    # admit the (16 - q) lowest-index ==V elements
                    rq = spool.tile([P, 1], F32, tag="rq")
                    nc.vector.tensor_scalar(rq[:], q[:], -1.0, float(K),
                                            op0=ALU.mult, op1=ALU.add)
                    t3 = spool.tile([P, K], F32, tag="t3")
                    nc.vector.tensor_scalar(t3[:], iota16_f[:], rq[:], None,
                                            op0=ALU.is_lt)
                    t4 = spool.tile([P, K], F32, tag="t4")
                    nc.vector.tensor_tensor(t4[:], t3[:], xs[:], op=ALU.mult)
                    xthr = spool.tile([P, 1], F32, tag="xthr")
                    nc.vector.tensor_reduce(xthr[:], t4[:],
                                            axis=mybir.AxisListType.X,
                                            op=ALU.max)
                    t5 = spool.tile([P, GW], F32, tag="t5")
                    nc.gpsimd.tensor_scalar(t5[:], X[:], xthr[:], None,
                                            op0=ALU.is_le)
                    selEQ = spool.tile([P, GW], F32, tag="selEQ")
                    nc.gpsimd.tensor_tensor(selEQ[:], t5[:], maskEQ[:],
                                            op=ALU.mult)
                    sel = spool.tile([P, GW], F32, tag="sel")
                    nc.gpsimd.tensor_tensor(sel[:], maskLT[:], selEQ[:],
                                            op=ALU.add)

                    # weights 1/d with zero-distance row fixup
                    Gsafe = spool.tile([P, GW], F32, tag="Gsafe")
                    nc.gpsimd.tensor_scalar(
                        Gsafe[:].rearrange("p (c t) -> p c t", t=CH),
                        Gf3, zr[:], None, op0=ALU.add)
                    wb = spool.tile([P, GW], F32, tag="wb")
                    wbs = spool.tile([P, GW], F32, tag="wbs")
                    nc.vector.reciprocal_approx_accurate(wb[:], Gsafe[:],
                                                         wbs[:])
                    # t7 = (Gf == 0) * zr  in one gpsimd op
                    t7 = spool.tile([P, GW], F32, tag="t7")
                    nc.gpsimd.tensor_scalar(
                        t7[:].rearrange("p (c t) -> p c t", t=CH),
                        Gf3, 0.0, zr[:], op0=ALU.is_equal, op1=ALU.mult)
                    t6 = spool.tile([P, GW], F32, tag="t6")
                    nc.scalar.activation(t6[:], wb[:], ACTF.Copy, scale=zc[:])
                    wfin = spool.tile([P, GW], F32, tag="wfin")
                    nc.gpsimd.tensor_tensor(wfin[:], t6[:], t7[:], op=ALU.add)
                    wsel = spool.tile([P, GW], F32, tag="wsel")
                    nc.vector.tensor_tensor(wsel[:], wfin[:], sel[:],
                                            op=ALU.mult)
                    if debug_dump and b == 0:
                        nc.sync.dma_start(dbg["X_dbg"].ap(), X[:])
                        nc.sync.dma_start(dbg["sel_dbg"].ap(), sel[:])
                        nc.sync.dma_start(dbg["wsel_dbg"].ap(), wsel[:])

                    # masked weighted mean
                    junk1 = spool.tile([P, GW], F32, tag="junk1")
                    num = spool.tile([P, 1], F32, tag="num")
                    nc.vector.scalar_tensor_tensor(
                        junk1[:].rearrange("p (c t) -> p c t", t=CH),
                        wsel[:].rearrange("p (c t) -> p c t", t=CH), 1.0,
                        GXB[:, :, CH:2 * CH],
                        op0=ALU.mult, op1=ALU.mult, accum_out=num[:])
                    junk2 = spool.tile([P, GW], F32, tag="junk2")
                    den = spool.tile([P, 1], F32, tag="den")
                    nc.vector.scalar_tensor_tensor(
                        junk2[:].rearrange("p (c t) -> p c t", t=CH),
                        wsel[:].rearrange("p (c t) -> p c t", t=CH), 1.0,
                        GXB[:, :, 2 * CH:3 * CH],
                        op0=ALU.mult, op1=ALU.mult, accum_out=den[:])

                    den0 = spool.tile([P, 1], F32, tag="den0")
                    nc.vector.scalar_tensor_tensor(
                        den0[:], den[:], 0.0, den[:],
                        op0=ALU.is_equal, op1=ALU.add)
                    rden = spool.tile([P, 1], F32, tag="rden")
                    nc.vector.reciprocal(rden[:], den0[:])
                    nc.vector.tensor_tensor(res_acc[:, b:b + 1], num[:],
                                            rden[:], op=ALU.mult)

                # single DMA out: res_acc[p, b] -> res[b*P + p]
                nc.sync.dma_start(
                    res_dram.ap().rearrange("(b p) one -> p (b one)", p=P),
                    res_acc[:])
            if loop_ctx is not None:
                loop_ctx.__exit__(None, None, None)

    nc.compile()
    return nc


_module_cache = {}


def _get_module(rep: int = 1):
    if rep not in _module_cache:
        _module_cache[rep] = build_module(rep)
    return _module_cache[rep]


def _prep_inputs(dist_pot_donors, fit_X_col, mask_fit_X_col):
    d = np.ascontiguousarray(np.asarray(dist_pot_donors, dtype=np.float32))
    assert d.shape == (R_FULL, D), d.shape
    x = np.asarray(fit_X_col, dtype=np.float32).reshape(D)
    m = np.asarray(mask_fit_X_col).reshape(D)
    mb = (1 - m).astype(np.float32)
    xm = (x * mb).astype(np.float32)
    # combined gather table: [d-chunk | xm-chunk | mb-chunk] per (row,chunk)
    comb = np.empty((R_FULL * NCHUNK, 3 * CH), dtype=np.float32)
    comb[:, 0:CH] = d.reshape(-1, CH)
    comb[:, CH:2 * CH] = np.tile(xm.reshape(NCHUNK, CH), (R_FULL, 1))
    comb[:, 2 * CH:3 * CH] = np.tile(mb.reshape(NCHUNK, CH), (R_FULL, 1))
    in_maps = [{"d": d[c * RPC:(c + 1) * RPC],
                "comb": comb[c * RPC * NCHUNK:(c + 1) * RPC * NCHUNK]}
               for c in range(N_CORES)]
    return in_maps


def kernel(dist_pot_donors, n_neighbors, fit_X_col, mask_fit_X_col):
    assert int(n_neighbors) == K, n_neighbors
    in_maps = _prep_inputs(dist_pot_donors, fit_X_col, mask_fit_X_col)
    nc = _get_module()
    r = bass_utils.run_bass_kernel_spmd(nc, in_maps,
                                        core_ids=list(range(N_CORES)))
    out = np.concatenate([r.results[c]["res"].reshape(RPC)
                          for c in range(N_CORES)])
    return out.astype(np.float32)
